# revision 2
# baseline (speedup 1.0000x reference)
"""BatchBlur: depthwise 15x15 conv with per-sample kernels, reflection pad 7.

x: (32, 3, 512, 512) f32, kernel: (32, 15, 15) f32 -> out (32, 3, 512, 512) f32.

Strategy: pure data parallel over batch, 4 samples (12 channel-images) per
core on 8 cores. Host: reflection-pad x to (., 526, 526), cast to fp16, and
build banded matrices A[s, k, j, m] = kern[s, k-m, j] (0 <= k-m < 15,
k < 46), duplicated at partition offset 64 for the upper row-half.

Device: the PE array runs in 64x32 tiling mode -> 8 independent tiles
(row half h in {0,64} x column tile c in {0,32,64,96}), each streaming its
own 46x512 matmul concurrently. Tile (h,c) processes 32-row output strips
of sample c's images; one matmul per horizontal tap j (15 accumulating
matmuls per strip):
  out[m, n] += sum_i A[i, m] * xp[r0+i, n+j],  A[i, m] = kern[i-m, j]
526 = 16*32 + 14, so 16 uniform strips cover an image exactly, each strip's
46 input rows are loaded once (single band - no shifted copies), and all
rhs column slices [j, j+512) stay in bounds. PSUM: one f32 bank per
(step, row-half), all 128 partitions used (4 column tiles x 32 rows).
Output is evicted to fp16 (DVE cast) and stored as fp16, halving store
bytes; the host casts back to f32 (adds ~4e-4 relative error, fine).
"""
import os
import sys

for _p in ("/opt/trn_rl_repo", "/root/.axon_site/_ro/trn_rl_repo"):
    if _p not in sys.path and os.path.isdir(_p):
        sys.path.insert(0, _p)

import numpy as np

import concourse.bass as bass
import concourse.mybir as mybir
import concourse.tile as tile
from concourse import bacc
from concourse.bass_utils import run_bass_kernel_spmd

L = 15           # blur kernel size
P = L // 2       # reflection pad
B, C, H, W = 32, 3, 512, 512
N_CORES = 8
BS = B // N_CORES            # samples per core (4)
NIMG = BS * C                # channel images per core (12)
HP, WP = H + 2 * P, W + 2 * P  # 526
M = 32                       # output rows per strip (column-tile width)
KG = M + L - 1               # 46 input rows per strip (row-tile K)
NSTRIP = H // M              # 16 strips per image, exact: HP = NSTRIP*M + L-1
QB = 4                       # strips per load/store batch
NBATCH = NIMG * NSTRIP // (8 * QB)  # 6 batches per (h, c) tile
N_WARMUP = 100               # dummy matmuls to release the HAM clock gate

F16 = mybir.dt.float16
F32 = mybir.dt.float32

_program_cache = None


def _batches(h: int, c: int):
    """Per-PE-tile work list: NBATCH batches of (img, first_strip), each
    covering QB consecutive 32-row strips of one image. Column c handles
    sample c's three channel-images 3c..3c+2; row-half 0 takes the first
    24 strips (img 3c all, img 3c+1 strips 0-7), half 1 the rest."""
    units = [(3 * c + i // NSTRIP, i % NSTRIP) for i in range(3 * NSTRIP)]
    mine = units[24 * h:24 * (h + 1)]
    return [mine[QB * b] for b in range(NBATCH)]


def _build_program():
    nc = bacc.Bacc("TRN2", target_bir_lowering=False, debug=False)
    xp_d = nc.dram_tensor("xp", [NIMG, HP, WP], F16, kind="ExternalInput").ap()
    a_d = nc.dram_tensor("a", [BS, 128, L, M], F16,
                         kind="ExternalInput").ap()
    out_d = nc.dram_tensor("out", [NIMG, H, W], F16,
                           kind="ExternalOutput").ap()

    with tile.TileContext(nc) as tc:
        with (
            tc.tile_pool(name="aconst", bufs=1) as apool,
            tc.tile_pool(name="warm", bufs=1) as wpool,
            tc.tile_pool(name="xin", bufs=2) as xpool,
            tc.tile_pool(name="oout", bufs=2) as opool,
            tc.tile_pool(name="psum", bufs=3, space="PSUM") as psum,
            tc.tile_pool(name="psumw", bufs=1, space="PSUM") as psumw,
        ):
            # HAM warm-up: a burst of full-width matmuls on a zeroed scratch
            # tile releases the PE clock gate while the first input DMAs are
            # in flight.
            wsrc = wpool.tile([128, 64], mybir.dt.bfloat16)
            nc.gpsimd.memset(wsrc[:], 0.0)
            wacc = psumw.tile([64, 64], F32)
            for _ in range(N_WARMUP):
                nc.tensor.matmul(wacc[:], wsrc[:, :64], wsrc[:], start=True,
                                 stop=True)

            def load_batch(t, h, img, strip0):
                # one DMA brings QB strips' worth of rows (overlapping
                # strided read, stride 32 rows) into partitions 64h..64h+46,
                # free-dim blocks q=0..3 of WP columns each
                base = (img * HP + strip0 * M) * WP
                q = nc.sync if h == 0 else nc.gpsimd
                q.dma_start(
                    out=t[64 * h:64 * h + KG, :].rearrange(
                        "p (q c) -> p q c", c=WP),
                    in_=bass.AP(xp_d.tensor, base,
                                [[WP, KG], [M * WP, QB], [1, WP]]))

            # first batch of all 8 PE tiles, then the per-sample A matrices
            xt = {}
            for h in range(2):
                for c in range(4):
                    img, s0 = _batches(h, c)[0]
                    t = xpool.tile([128, QB * WP], F16, tag=f"x{h}{c}",
                                   name=f"x{h}{c}")
                    load_batch(t, h, img, s0)
                    xt[(h, c)] = t

            a_t = [
                apool.tile([128, L, M], F16, tag=f"a{s}", name=f"a{s}")
                for s in range(BS)
            ]
            for s in range(BS):
                nc.sync.dma_start(out=a_t[s][:], in_=a_d[s])

            o_t = {}
            for step in range(NBATCH * QB):
                b, q = step // QB, step % QB
                if q == 0:
                    # prefetch next batch; allocate this batch's output tile
                    for h in range(2):
                        for c in range(4):
                            if b + 1 < NBATCH:
                                img, s0 = _batches(h, c)[b + 1]
                                t = xpool.tile([128, QB * WP], F16,
                                               tag=f"x{h}{c}", name=f"xn{h}{c}")
                                load_batch(t, h, img, s0)
                                xt[(h, c, b + 1)] = t
                    for h in range(2):
                        o_t[h] = opool.tile([128, QB * W], F16, tag=f"o{h}",
                                            name=f"o{h}")

                acc = [psum.tile([128, W], F32, tag=f"ps{h}", name=f"ps{h}")
                       for h in range(2)]
                for j in range(L):
                    for h in range(2):
                        for c in range(4):
                            nc.tensor.matmul(
                                acc[h][32 * c:32 * c + M, :],
                                a_t[c][64 * h:64 * h + KG, j, :],
                                xt[(h, c)][64 * h:64 * h + KG,
                                           WP * q + j:WP * q + j + W],
                                start=(j == 0),
                                stop=(j == L - 1),
                                tile_position=(64 * h, 32 * c),
                            )
                for h in range(2):
                    nc.vector.tensor_copy(out=o_t[h][:, q * W:(q + 1) * W],
                                          in_=acc[h][:])

                if q == QB - 1:
                    # store QB strips per (h, c): 128 contiguous output rows
                    for h in range(2):
                        for c in range(4):
                            img, s0 = _batches(h, c)[b]
                            dv = out_d[img, s0 * M:s0 * M + QB * M,
                                       :].rearrange("(q p) c -> p q c", q=QB)
                            sv = o_t[h][32 * c:32 * c + M, :].rearrange(
                                "p (q c) -> p q c", c=W)
                            nc.scalar.dma_start(out=dv, in_=sv)
                    # swap in the prefetched batch
                    for h in range(2):
                        for c in range(4):
                            if b + 1 < NBATCH:
                                xt[(h, c)] = xt.pop((h, c, b + 1))
    nc.compile()
    return nc


def prepare_in_maps(x: np.ndarray, kern: np.ndarray) -> list:
    # host-side reflection pad, cast to fp16 for half the DMA bytes
    xp = np.pad(x, ((0, 0), (0, 0), (P, P), (P, P)), mode="reflect")
    xp = np.ascontiguousarray(
        xp.reshape(B * C, HP, WP).astype(np.float16))

    # banded matrices A[s, k, j, m] = kern[s, k-m, j], duplicated at
    # partition offset 64 for the upper row-half of the PE array
    kern16 = kern.astype(np.float16)
    a_all = np.zeros((B, 128, L, M), dtype=np.float16)
    m_idx = np.arange(M)
    for dy in range(L):
        a_all[:, m_idx + dy, :, m_idx] = kern16[:, dy, :]
    a_all[:, 64:64 + KG] = a_all[:, 0:KG]

    return [
        {
            "xp": xp[c * NIMG:(c + 1) * NIMG],
            "a": a_all[c * BS:(c + 1) * BS],
        }
        for c in range(N_CORES)
    ]


def kernel(x: np.ndarray, kernel: np.ndarray) -> np.ndarray:
    global _program_cache
    x = np.asarray(x, dtype=np.float32)
    kern = np.asarray(kernel, dtype=np.float32)

    in_maps = prepare_in_maps(x, kern)
    if _program_cache is None:
        _program_cache = _build_program()
    nc = _program_cache

    res = run_bass_kernel_spmd(nc, in_maps, core_ids=list(range(N_CORES)))
    out = np.concatenate([r["out"] for r in res.results], axis=0)
    return out.reshape(B, C, H, W).astype(np.float32)
